# revision 1
# baseline (speedup 1.0000x reference)
"""Bass/Trainium2 kernel for nn_HE_FM (factorization machine embedding lookup).

Computation: out[n] = W[uid[n]] + W[iid[n]+USER_NUM] + b + dot(V[uid[n]], V[iid[n]+USER_NUM])

Strategy (data parallel over batch, tables replicated on all 8 cores):
  - Host builds an augmented table A [1.5M, 66] f32:
      user rows  (r < 1M):  A[r] = [V[r], W[r], 1.0]
      item rows  (r >= 1M): A[r] = [V[r], 1.0, W[r]+b]
    so dot(A[uid], A[iid+1M]) over 66 elements equals the full FM output.
  - Each core gathers 2*2048 rows of A with indirect (SWDGE) DMA,
    multiplies elementwise and does a segmented reduce of width 66.
"""

from contextlib import ExitStack

import numpy as np

import concourse.bass as bass
import concourse.mybir as mybir
from concourse.bass_utils import run_bass_kernel_spmd

USER_NUM = 1_000_000
ITEM_NUM = 500_000
TOTAL_ROWS = USER_NUM + ITEM_NUM
D = 64
WIDTH = D + 2  # V row + [W, 1] / [1, W+b]
BATCH = 16384
N_CORES = 8
B_CORE = BATCH // N_CORES  # 2048
P = 128


def build_program(total_rows=TOTAL_ROWS, user_num=USER_NUM, b_core=B_CORE):
    """Per-core SPMD program. Parameterized so tests can build a small variant."""
    k = b_core // P  # index columns per partition
    nc = bass.Bass()
    ids = nc.declare_dram_parameter("ids", [2, b_core], mybir.dt.int32, isOutput=False)
    table = nc.declare_dram_parameter(
        "table", [total_rows, WIDTH], mybir.dt.float32, isOutput=False
    )
    outp = nc.declare_dram_parameter("out", [b_core], mybir.dt.float32, isOutput=True)

    with (
        ExitStack() as ctx,
        nc.sbuf_tensor([P, k], mybir.dt.int32) as idx_u,
        nc.sbuf_tensor([P, k], mybir.dt.int32) as idx_i,
        nc.sbuf_tensor([P, k * WIDTH], mybir.dt.float32) as au,
        nc.sbuf_tensor([P, k * WIDTH], mybir.dt.float32) as ai,
        nc.sbuf_tensor([P, k * WIDTH], mybir.dt.float32) as prod,
        nc.sbuf_tensor([P, k], mybir.dt.float32) as resv,
        nc.Block() as block,
        nc.semaphore("iu_sem") as iu_sem,
        nc.semaphore("ii_sem") as ii_sem,
        nc.semaphore("o_sem") as o_sem,
        nc.semaphore("tt_sem") as tt_sem,
        nc.semaphore("v_sem") as v_sem,
    ):
        # One semaphore per column pair: DMA-completion increments from
        # different instructions interleave across the 16 SDMA engines, so
        # only a semaphore's full value is a sound wait point.
        g_sems = [ctx.enter_context(nc.semaphore(f"g_sem{j}")) for j in range(k)]

        @block.sync
        def _(sync: bass.BassEngine):
            sync.dma_start(
                out=idx_u[:], in_=ids[0].rearrange("(p k) -> p k", p=P)
            ).then_inc(iu_sem, 16)
            sync.dma_start(
                out=idx_i[:], in_=ids[1].rearrange("(p k) -> p k", p=P)
            ).then_inc(ii_sem, 16)
            sync.wait_ge(v_sem, k)
            sync.dma_start(
                out=outp[:].rearrange("(p k) -> p k", p=P), in_=resv[:]
            ).then_inc(o_sem, 16)
            sync.wait_ge(o_sem, 16)

        @block.gpsimd
        def _(gpsimd: bass.BassEngine):
            # HW indirect DMA: one descriptor per partition per instruction,
            # using idx[p, 0] — so one gather instruction per index column.
            # Interleave u/i columns so DVE can start on column j as soon as
            # its pair of gathers lands. u-gathers only need idx_u, so the
            # first gather starts as soon as that 8KB DMA completes.
            gpsimd.wait_ge(iu_sem, 16)
            gpsimd.indirect_dma_start(
                out=au[:, 0:WIDTH],
                out_offset=None,
                in_=table[:],
                in_offset=bass.IndirectOffsetOnAxis(ap=idx_u[:, 0:1], axis=0),
            ).then_inc(g_sems[0], 16)
            gpsimd.wait_ge(ii_sem, 16)
            gpsimd.indirect_dma_start(
                out=ai[:, 0:WIDTH],
                out_offset=None,
                in_=table[:],
                in_offset=bass.IndirectOffsetOnAxis(ap=idx_i[:, 0:1], axis=0),
                element_offset=user_num * WIDTH,
            ).then_inc(g_sems[0], 16)
            for j in range(1, k):
                gpsimd.indirect_dma_start(
                    out=au[:, j * WIDTH : (j + 1) * WIDTH],
                    out_offset=None,
                    in_=table[:],
                    in_offset=bass.IndirectOffsetOnAxis(ap=idx_u[:, j : j + 1], axis=0),
                ).then_inc(g_sems[j], 16)
                gpsimd.indirect_dma_start(
                    out=ai[:, j * WIDTH : (j + 1) * WIDTH],
                    out_offset=None,
                    in_=table[:],
                    in_offset=bass.IndirectOffsetOnAxis(ap=idx_i[:, j : j + 1], axis=0),
                    element_offset=user_num * WIDTH,
                ).then_inc(g_sems[j], 16)

        @block.vector
        def _(vector: bass.BassEngine):
            for j in range(k):
                vector.wait_ge(g_sems[j], 32)
                vector.tensor_tensor(
                    out=prod[:, j * WIDTH : (j + 1) * WIDTH],
                    in0=au[:, j * WIDTH : (j + 1) * WIDTH],
                    in1=ai[:, j * WIDTH : (j + 1) * WIDTH],
                    op=mybir.AluOpType.mult,
                ).then_inc(tt_sem, 1)
                vector.wait_ge(tt_sem, j + 1)
                vector.tensor_reduce(
                    out=resv[:, j : j + 1],
                    in_=prod[:, j * WIDTH : (j + 1) * WIDTH].rearrange(
                        "p (k w) -> p k w", w=WIDTH
                    ),
                    axis=mybir.AxisListType.X,
                    op=mybir.AluOpType.add,
                ).then_inc(v_sem, 1)

    _strip_dead_const_memsets(nc)
    return nc


def _strip_dead_const_memsets(nc):
    """Bass.__init__ unconditionally memsets four const-* SBUF tensors on
    gpsimd; this kernel never reads them (birverifier agrees: "no reader"),
    and they sit on the Pool critical path ahead of the gathers."""
    for bb in nc.m.functions[0].blocks:
        keep = []
        for inst in bb.instructions:
            is_dead_const = type(inst).__name__ == "InstMemset" and any(
                getattr(out, "memref", "").startswith("const-") for out in inst.outs
            )
            if not is_dead_const:
                keep.append(inst)
        if len(keep) != len(bb.instructions):
            bb.instructions[:] = keep


def build_table(W, b, V, total_rows=TOTAL_ROWS, user_num=USER_NUM):
    A = np.empty((total_rows, WIDTH), dtype=np.float32)
    A[:, :D] = V
    A[:user_num, D] = W[:user_num, 0]
    A[:user_num, D + 1] = 1.0
    A[user_num:, D] = 1.0
    A[user_num:, D + 1] = W[user_num:, 0] + b[0]
    return A


_program_cache = {}


def kernel(INPUT, W, b, V):
    INPUT = np.asarray(INPUT, dtype=np.int32)
    W = np.asarray(W, dtype=np.float32)
    b = np.asarray(b, dtype=np.float32)
    V = np.asarray(V, dtype=np.float32)

    if "nc" not in _program_cache:
        _program_cache["nc"] = build_program()
    nc = _program_cache["nc"]

    A = build_table(W, b, V)
    # ids[i] : [2, B_CORE] int32 — row 0 = uid, row 1 = raw iid (+USER_NUM on device)
    ids = np.ascontiguousarray(
        INPUT.reshape(N_CORES, B_CORE, 2).transpose(0, 2, 1)
    ).astype(np.int32)

    in_maps = [{"ids": ids[i], "table": A} for i in range(N_CORES)]
    res = run_bass_kernel_spmd(nc, in_maps, core_ids=list(range(N_CORES)))
    global last_results
    last_results = res
    out = np.concatenate([np.asarray(res.results[i]["out"]) for i in range(N_CORES)])
    return out.reshape(BATCH, 1).astype(np.float32)


last_results = None



# revision 8
# speedup vs baseline: 1.0227x; 1.0227x over previous
"""Bass/Trainium2 kernel for nn_HE_FM (factorization machine embedding lookup), v3.

Computation: out[n] = W[uid[n]] + W[iid[n]+USER_NUM] + b + dot(V[uid[n]], V[iid[n]+USER_NUM])

Strategy (data parallel over batch, fp16 augmented table replicated on 8 cores):
  - Host builds an augmented table A [1.5M, 66] fp16:
      user rows  (r < 1M):  A[r] = [V[r], W[r], 1.0]
      item rows  (r >= 1M): A[r] = [V[r], 1.0, W[r]+b]
    so dot(A[uid], A[iid+1M]) over 66 elements equals the full FM output.
  - Each core gathers 2*2048 rows of A with indirect (SWDGE) DMA; the backend
    supports one dynamic row per partition per instruction, so 32 gathers
    (Pool-engine fixed cost is the structural floor: ~994ns per SWDGE DMA).
  - DVE does tensor_tensor (mult) + tensor_reduce per column pair; in-order
    engine execution makes an intra-DVE semaphore unnecessary.
  - fp16 halves HBM gather traffic and DVE time; max rel err ~2.6e-4.
  - Framework preamble/epilogue trimmed (unused PE+Activation engines, unused
    bounds-check register init, post-output drain barrier): every DMA is
    semaphore-awaited before the program ends, so the teardown barrier only
    added serial time after the last store.
"""

from contextlib import ExitStack

import numpy as np

import concourse.bass as bass
import concourse.mybir as mybir
from concourse.bass_utils import run_bass_kernel_spmd

USER_NUM = 1_000_000
ITEM_NUM = 500_000
TOTAL_ROWS = USER_NUM + ITEM_NUM
D = 64
WIDTH = D + 2  # V row + [W, 1] / [1, W+b]
BATCH = 16384
N_CORES = 8
B_CORE = BATCH // N_CORES  # 2048
P = 128
K = B_CORE // P  # 16 column pairs per partition

STRIP_PE_PREAMBLE = True
STRIP_BCREGS = True
STRIP_EPILOGUE = True


def build_program(total_rows=TOTAL_ROWS, dtype=mybir.dt.float16):
    nc = bass.Bass()
    ids = nc.declare_dram_parameter("ids", [P, 2 * K], mybir.dt.int32, isOutput=False)
    table = nc.declare_dram_parameter("table", [total_rows, WIDTH], dtype, isOutput=False)
    outp = nc.declare_dram_parameter("out", [B_CORE], mybir.dt.float32, isOutput=True)

    with (
        ExitStack() as ctx,
        nc.sbuf_tensor([P, 2 * K], mybir.dt.int32) as idx,
        nc.sbuf_tensor([P, 2 * K * WIDTH], dtype) as au,
        nc.sbuf_tensor([P, WIDTH], dtype) as prod,
        nc.sbuf_tensor([P, K], mybir.dt.float32) as resv,
        nc.Block() as block,
        nc.semaphore("i_sem") as i_sem,
        nc.semaphore("v_sem") as v_sem,
        nc.semaphore("o_sem") as o_sem,
    ):
        g_sems = [ctx.enter_context(nc.semaphore(f"g_sem{j}")) for j in range(K)]

        @block.sync
        def _(sync: bass.BassEngine):
            # idx layout: columns 2j = uid, 2j+1 = iid+USER_NUM for pair j.
            sync.dma_start(out=idx[:], in_=ids[:]).then_inc(i_sem, 16)
            # bulk store of pairs 0..K-2 once they are done, then the last
            # pair alone so the final DMA's payload (and exposure) is minimal.
            sync.wait_ge(v_sem, K - 1)
            sync.dma_start(
                out=outp[:].rearrange("(p k) -> p k", p=P)[:, 0 : K - 1],
                in_=resv[:, 0 : K - 1],
            ).then_inc(o_sem, 16)
            sync.wait_ge(v_sem, K)
            with nc.allow_non_contiguous_dma(
                reason="single-column tail store: 128 x 4B at stride 64B"
            ):
                sync.dma_start(
                    out=outp[:].rearrange("(p k) -> p k", p=P)[:, K - 1 : K],
                    in_=resv[:, K - 1 : K],
                ).then_inc(o_sem, 16)
            sync.wait_ge(o_sem, 32)

        @block.gpsimd
        def _(gpsimd: bass.BassEngine):
            gpsimd.wait_ge(i_sem, 16)
            for j in range(K):
                for t in range(2):
                    c = 2 * j + t
                    gpsimd.indirect_dma_start(
                        out=au[:, c * WIDTH : (c + 1) * WIDTH],
                        out_offset=None,
                        in_=table[:],
                        in_offset=bass.IndirectOffsetOnAxis(
                            ap=idx[:, c : c + 1], axis=0
                        ),
                    ).then_inc(g_sems[j], 16)

        @block.vector
        def _(vector: bass.BassEngine):
            # tensor_tensor then tensor_reduce per pair; no semaphore between
            # them — the DVE executes its queue in order, so the RAW on prod
            # is satisfied by program order.
            for j in range(K):
                vector.wait_ge(g_sems[j], 32)
                vector.tensor_tensor(
                    out=prod[:],
                    in0=au[:, (2 * j) * WIDTH : (2 * j + 1) * WIDTH],
                    in1=au[:, (2 * j + 1) * WIDTH : (2 * j + 2) * WIDTH],
                    op=mybir.AluOpType.mult,
                )
                vector.tensor_reduce(
                    out=resv[:, j : j + 1],
                    in_=prod[:].rearrange("p (k w) -> p k w", w=WIDTH),
                    axis=mybir.AxisListType.X,
                    op=mybir.AluOpType.add,
                ).then_inc(v_sem, 1)

    _strip_dead_const_memsets(nc)
    if STRIP_PE_PREAMBLE:
        _strip_pe_preamble(nc)
    if STRIP_BCREGS:
        _strip_bcreg_moves(nc)
    if STRIP_EPILOGUE:
        _strip_epilogue_barrier(nc)
    return nc


def _strip_epilogue_barrier(nc):
    """Remove the final all-engine drain barrier. Every DMA this kernel issues
    is awaited via its semaphore before the program ends (gathers by DVE, idx
    by Pool, stores by SP's o_sem wait), so the teardown barrier only adds
    serial time after the last store completes."""
    fn = nc.m.functions[0]
    last = fn.blocks[-1]
    only = {"InstDrain", "InstEventSemaphore", "InstHalt", "InstNoOp"}
    if all(type(i).__name__ in only for i in last.instructions):
        last.instructions[:] = [
            i for i in last.instructions if type(i).__name__ == "InstHalt"
        ]


def _strip_bcreg_moves(nc):
    """The preamble initializes bounds-check registers (*_bcreg{0,1}_{lo,hi})
    on every engine; no DMA in this kernel uses bounds_check, and the moves
    delay each engine's arrival at the startup barrier (which gates the idx
    DMA). Keep *_zero and monotonic counters."""
    fn = nc.m.functions[0]
    for bb in fn.blocks:
        keep = []
        for inst in bb.instructions:
            if type(inst).__name__ == "InstRegisterMove" and any(
                "_bcreg" in (getattr(o, "regref", "") or "") for o in (inst.outs or [])
            ):
                continue
            keep.append(inst)
        if len(keep) != len(bb.instructions):
            bb.instructions[:] = keep


def _strip_dead_const_memsets(nc):
    """Bass.__init__ unconditionally memsets four const-* SBUF tensors on
    gpsimd; this kernel never reads them, and they sit on the Pool critical
    path ahead of the gathers."""
    for bb in nc.m.functions[0].blocks:
        keep = []
        for inst in bb.instructions:
            is_dead_const = type(inst).__name__ == "InstMemset" and any(
                getattr(out, "memref", "").startswith("const-") for out in inst.outs
            )
            if not is_dead_const:
                keep.append(inst)
        if len(keep) != len(bb.instructions):
            bb.instructions[:] = keep


def _strip_pe_preamble(nc):
    """The framework pre/postamble runs an all-engine barrier (each non-Pool
    engine Drains then incs the 'gather' sem; Pool waits gather>=4 and releases
    everyone). PE and Activation are unused by this kernel but PE is the
    slowest to arrive, gating SP's first DMA. Remove both engines' (purely
    preamble) instructions and lower the Pool-side barrier counts 4 -> 2."""
    import concourse.mybir as mb

    strip = {mb.EngineType.PE, mb.EngineType.Activation}
    fn = nc.m.functions[0]
    n_stripped_engines = 2

    for bb in fn.blocks:
        removed = [
            i for i in bb.instructions if getattr(i, "engine", None) in strip
        ]
        # Safety: only preamble instruction types may be dropped.
        for i in removed:
            assert type(i).__name__ in (
                "InstRegisterMove",
                "InstDrain",
                "InstEventSemaphore",
                "InstUnconditionalBranch",
                "InstNoOp",
                "InstHalt",
            ), f"unexpected {type(i).__name__} on stripped engine"
        if removed:
            bb.instructions[:] = [
                i for i in bb.instructions if getattr(i, "engine", None) not in strip
            ]

    # Patch the Pool-side barrier arithmetic.
    for bb in fn.blocks:
        for inst in bb.instructions:
            si = getattr(inst, "sync_info", None)
            if si is None:
                continue
            for w in si.on_wait or []:
                if "barrier_" in (w.ant_name or "") and w.ant_name.endswith("_gather"):
                    w.wait_value = w.wait_value - n_stripped_engines
            for u in si.on_update or []:
                name = u.ant_name or ""
                if "barrier_" in name and (
                    name.endswith("_gather") or name.endswith("_release")
                ):
                    if u.update_mode in ("sem-sub-imm", "sem-add-imm"):
                        u.update_value = u.update_value - n_stripped_engines


def build_table(W, b, V, total_rows=TOTAL_ROWS, user_num=USER_NUM):
    A = np.empty((total_rows, WIDTH), dtype=np.float16)
    A[:, :D] = V
    A[:user_num, D] = W[:user_num, 0]
    A[:user_num, D + 1] = 1.0
    A[user_num:, D] = 1.0
    A[user_num:, D + 1] = W[user_num:, 0] + b[0]
    return A


def build_ids(INPUT):
    """ids[core] : [P, 2K] int32; column 2j = uid, 2j+1 = iid+USER_NUM of
    batch element p*K + j."""
    uid = INPUT[:, 0].reshape(N_CORES, P, K)
    iid = (INPUT[:, 1] + USER_NUM).reshape(N_CORES, P, K)
    ids = np.empty((N_CORES, P, 2 * K), dtype=np.int32)
    ids[:, :, 0::2] = uid
    ids[:, :, 1::2] = iid
    return ids


_program_cache = {}


def kernel(INPUT, W, b, V):
    INPUT = np.asarray(INPUT, dtype=np.int32)
    W = np.asarray(W, dtype=np.float32)
    b = np.asarray(b, dtype=np.float32)
    V = np.asarray(V, dtype=np.float32)

    if "nc" not in _program_cache:
        _program_cache["nc"] = build_program()
    nc = _program_cache["nc"]

    A = build_table(W, b, V)
    ids = build_ids(INPUT)

    in_maps = [{"ids": ids[i], "table": A} for i in range(N_CORES)]
    res = run_bass_kernel_spmd(nc, in_maps, core_ids=list(range(N_CORES)))
    global last_results
    last_results = res
    out = np.concatenate([np.asarray(res.results[i]["out"]) for i in range(N_CORES)])
    return out.reshape(BATCH, 1).astype(np.float32)


last_results = None
